# revision 23
# baseline (speedup 1.0000x reference)
"""Fused pre-norm attention kernel for Trainium2, sharded over 8 NeuronCores.

Problem: out = (LayerNorm(x) @ w_qkv -> multi-head attention) @ w_out
  x [4, 2048, 512], 8 heads x 64 dim, fp32.

Sharding: core c computes batch b = c//2 with head group g = c%2 (4 heads).
Each core produces a partial output [2048, 512] (its heads' contribution to
the out-projection); the host sums the two partials per batch.

Per-core kernel (all SBUF-resident, flash-style, no score materialization
to HBM):
  1. LayerNorm x_b in token-major layout (bn_stats), gamma/beta folded into
     the QKV weights on the host.
  2. PE-transpose xn -> xnT [512, 2048] (feature-major).
  3. QKV: qT,kT = w'.T @ xnT (feature-contraction, weights as lhsT);
     v in token-major via xnT as lhsT. A ones-column is appended to v per
     head so the P@V matmul also produces the softmax denominator.
  4. Attention in S^T layout: S^T[k,q] = K @ Q^T block; exp on ACT engine
     (scale folded in, no max subtraction -- scores are bounded ~|9|);
     O^T[d,q] accumulates over key blocks on PE. Row 64 of the O^T psum is
     the softmax denominator; divide via reciprocal + partition-broadcast.
  5. Out-projection: partial = O^T.T @ w_out_rows, DMA to DRAM.
"""

import os
import sys
from contextlib import ExitStack

import numpy as np

for _p in ("/opt/trn_rl_repo",):
    if _p not in sys.path and os.path.isdir(_p):
        sys.path.insert(0, _p)

import concourse.bacc as bacc
import concourse.bass as bass
import concourse.mybir as mybir
import concourse.tile as tile
from concourse.bass_utils import run_bass_kernel_spmd
from concourse.masks import make_identity

F32 = mybir.dt.float32
F32R = mybir.dt.float32r
AF = mybir.ActivationFunctionType

N_CORES = 8
B, N, D = 4, 2048, 512
H_PER_CORE = 4
DH = 64
GCOLS = H_PER_CORE * DH          # 256 columns per head-group
WCOLS = 3 * GCOLS                # 768 qkv columns per core
SCALE = DH ** -0.5
EPS = 1e-5
P = 128                          # SBUF partitions
NT = N // P                      # 16 token tiles
KT = D // P                      # 4 feature (contraction) tiles
QTW = 512                        # query-slice width for attention
NQT = N // QTW                   # 4 query slices

# matmul streaming dtype: float32r = fp32 data on the fast (1 cycle/row)
# PE path; float32 = exact but 4 cycles/row.
_MM_DT_NAME = os.environ.get("BASS_MM_DT", "f32r")
MM_DT = F32R if _MM_DT_NAME == "f32r" else F32


def _build_nc():
    nc = bacc.Bacc(None)
    x_d = nc.declare_dram_parameter("x", [N, D], F32, isOutput=False)
    wqkv_d = nc.declare_dram_parameter("wqkv", [D, WCOLS], MM_DT, isOutput=False)
    bqkv_d = nc.declare_dram_parameter("bqkv", [WCOLS, 1], F32, isOutput=False)
    wout_d = nc.declare_dram_parameter("wout", [GCOLS, D], MM_DT, isOutput=False)
    out_d = nc.declare_dram_parameter("out", [N, D], F32, isOutput=True)

    with tile.TileContext(nc, pool_alloc_mode="queue") as tc, ExitStack() as ctx:
        singles = ctx.enter_context(tc.tile_pool(name="singles", bufs=1))
        xin = ctx.enter_context(tc.tile_pool(name="xin", bufs=NT))
        stats = ctx.enter_context(tc.tile_pool(name="stats", bufs=4))
        pP = ctx.enter_context(tc.tile_pool(name="pP", bufs=3))
        smalls = ctx.enter_context(tc.tile_pool(name="smalls", bufs=3))
        outp = ctx.enter_context(tc.tile_pool(name="outp", bufs=3))
        psA = ctx.enter_context(tc.tile_pool(name="psA", bufs=2, space="PSUM"))
        psS = ctx.enter_context(tc.tile_pool(name="psS", bufs=3, space="PSUM"))
        psO = ctx.enter_context(tc.tile_pool(name="psO", bufs=2, space="PSUM"))
        dscr = ctx.enter_context(tc.tile_pool(name="dscr", bufs=3, space="DRAM"))

        ident = singles.tile([P, P], F32)
        make_identity(nc, ident)
        eps_sb = singles.tile([P, 1], F32)
        nc.vector.memset(eps_sb, EPS)

        # persistent SBUF tensors
        xT = singles.tile([P, KT, N], MM_DT)            # xn^T  [feat, token]
        qkT = singles.tile([P, 4, N], MM_DT)            # [qT(2 tiles), kT(2 tiles)]
        v_aug = singles.tile([P, NT, H_PER_CORE, DH + 1], MM_DT)
        oT = singles.tile([P, 2, N], MM_DT)             # O^T rows (4 heads x 64)
        w_sb = singles.tile([P, KT, WCOLS], MM_DT)
        bias_sb = singles.tile([P, 6], F32)
        vbias_sb = singles.tile([P, GCOLS], F32)
        wout_sb = singles.tile([P, 2, D], MM_DT)

        nc.sync.dma_start(out=w_sb, in_=wqkv_d[:, :].rearrange("(kt p) m -> p kt m", p=P))
        nc.sync.dma_start(out=bias_sb, in_=bqkv_d[:, :].rearrange("(t p) o -> p (t o)", p=P))
        nc.sync.dma_start(out=wout_sb, in_=wout_d[:, :].rearrange("(ki p) n -> p ki n", p=P))
        bq = bqkv_d[:, :]
        vbias_bcast = bass.AP(
            tensor=bq.tensor, offset=2 * GCOLS, ap=[[0, P], [1, GCOLS]]
        )
        nc.sync.dma_start(out=vbias_sb, in_=vbias_bcast)

        # ones columns of v_aug (f32r memset is not a valid ISA op; copy
        # from an f32 ones tile instead -- DVE rounds on write)
        ones_sb = singles.tile([P, 1], F32)
        nc.vector.memset(ones_sb, 1.0)
        nc.vector.tensor_copy(
            out=v_aug[:, :, :, DH : DH + 1],
            in_=ones_sb.to_broadcast((P, NT, H_PER_CORE, 1)),
        )

        # PE matmuls accept only ONE sync wait command (walrus
        # setupSyncWait on the S3_LW format). Sacrificial ldweights ops
        # (no PSUM output, single dependency each) make the PE observe
        # fresh semaphore ticks so real matmuls keep to one wait.
        BF16 = mybir.dt.bfloat16

        def pe_observe(ap):
            nc.tensor.ldweights(ap.bitcast(BF16))

        pe_observe(ident[:, 0:1])
        pe_observe(w_sb[:, 0, 0:1])
        pe_observe(wout_sb[:, 0, 0:1])

        # ---- Phase A: LayerNorm + transpose ----------------------------
        for tt in range(NT):
            x_tile = xin.tile([P, D], F32)
            nc.sync.dma_start(out=x_tile, in_=x_d[tt * P : (tt + 1) * P, :])
            st = stats.tile([P, nc.vector.BN_STATS_DIM], F32)
            nc.vector.bn_stats(out=st, in_=x_tile)
            mv = stats.tile([P, nc.vector.BN_AGGR_DIM], F32)
            nc.vector.bn_aggr(out=mv, in_=st)
            rstd = stats.tile([P, 1], F32)
            nc.scalar.activation(out=rstd, in_=mv[:, 1:2], func=AF.Sqrt, bias=eps_sb)
            nc.vector.reciprocal(out=rstd, in_=rstd)
            nc.vector.tensor_scalar(
                out=x_tile,
                in0=x_tile,
                scalar1=mv[:, 0:1],
                scalar2=rstd,
                op0=mybir.AluOpType.subtract,
                op1=mybir.AluOpType.mult,
            )
            for ft in range(KT):
                ps = psA.tile([P, P], F32)
                nc.tensor.transpose(ps, x_tile[:, ft * P : (ft + 1) * P], ident)
                nc.vector.tensor_copy(out=xT[:, ft, tt * P : (tt + 1) * P], in_=ps)

        # PE observes the final xT copy tick before QKV matmuls
        pe_observe(xT[:, KT - 1, N - 1 : N])

        # ---- Phase B: QKV projections ----------------------------------
        # v first (token-major, bias added, ones cols interleaved) so that
        # the q/k writes are the LAST DVE ticks PE waits on in phase C --
        # keeps every attention matmul at a single fresh semaphore wait.
        for tt in range(NT):
            if tt >= 2:
                # absorb the psA slot-release DVE tick (reader 2 groups ago)
                pe_observe(v_aug[:, tt - 2, 0, 0:1])
            ps = psA.tile([P, GCOLS], F32)
            for kt in range(KT):
                nc.tensor.matmul(
                    ps,
                    xT[:, kt, tt * P : (tt + 1) * P],
                    w_sb[:, kt, 2 * GCOLS : 3 * GCOLS],
                    start=(kt == 0),
                    stop=(kt == KT - 1),
                )
            nc.vector.tensor_add(
                out=v_aug[:, tt, :, 0:DH],
                in0=ps.rearrange("p (h d) -> p h d", h=H_PER_CORE),
                in1=vbias_sb.rearrange("p (h d) -> p h d", h=H_PER_CORE),
            )
        # q^T and k^T tiles: [cols(128), tokens] with cols as out partitions
        qk_hist = []
        for mi in range(4):  # 0,1 -> q col-tiles; 2,3 -> k col-tiles
            for nt in range(NQT):
                if len(qk_hist) >= 2:
                    m2, n2 = qk_hist[-2]
                    pe_observe(qkT[:, m2, n2 * QTW : n2 * QTW + 1])
                else:
                    # slots inherited from the last two v groups
                    pe_observe(v_aug[:, NT - 2 + len(qk_hist), 0, 0:1])
                qk_hist.append((mi, nt))
                ps = psA.tile([P, QTW], F32)
                for kt in range(KT):
                    nc.tensor.matmul(
                        ps,
                        w_sb[:, kt, mi * P : (mi + 1) * P],
                        xT[:, kt, nt * QTW : (nt + 1) * QTW],
                        start=(kt == 0),
                        stop=(kt == KT - 1),
                    )
                nc.vector.tensor_scalar(
                    out=qkT[:, mi, nt * QTW : (nt + 1) * QTW],
                    in0=ps,
                    scalar1=bias_sb[:, mi : mi + 1],
                    scalar2=None,
                    op0=mybir.AluOpType.add,
                )

        # PE observes the final qkT write tick (covers v_aug too)
        pe_observe(qkT[:, 3, N - 1 : N])

        # ---- Phase C: attention (S^T layout) ---------------------------
        for h in range(H_PER_CORE):
            mi_q = h // 2
            mi_k = 2 + h // 2
            r0 = (h % 2) * DH
            qT_h = qkT[r0 : r0 + DH, mi_q, :]
            kT_h = qkT[r0 : r0 + DH, mi_k, :]
            for qt in range(NQT):
                po = psO.tile([DH + 1, QTW], F32)
                pending = None  # (kb, pT) waiting for its O matmul
                for kb in range(NT):
                    ps_s = psS.tile([P, QTW], F32)
                    nc.tensor.matmul(
                        ps_s,
                        kT_h[:, kb * P : (kb + 1) * P],
                        qT_h[:, qt * QTW : (qt + 1) * QTW],
                        start=True,
                        stop=True,
                    )
                    pT = pP.tile([P, QTW], MM_DT)
                    nc.scalar.activation(out=pT, in_=ps_s, func=AF.Exp, scale=SCALE)
                    if pending is not None:
                        pkb, ppT = pending
                        if pkb == 0:
                            pe_observe(ppT[:, 0:1])
                        nc.tensor.matmul(
                            po,
                            v_aug[:, pkb, h, :],
                            ppT,
                            start=(pkb == 0),
                            stop=False,
                        )
                    pending = (kb, pT)
                pkb, ppT = pending
                nc.tensor.matmul(
                    po,
                    v_aug[:, pkb, h, :],
                    ppT,
                    start=False,
                    stop=True,
                )
                # normalize: row DH of po is the softmax denominator.
                # ACT copies PSUM->SBUF so the O matmuls' psO slot-release
                # dependency stays on the ACT semaphore (PE 1-wait limit).
                oc = smalls.tile([DH + 1, QTW], F32)
                nc.scalar.copy(out=oc, in_=po)
                r = smalls.tile([1, QTW], F32)
                nc.vector.reciprocal(out=r, in_=oc[DH : DH + 1, :])
                rd = dscr.tile([1, QTW], F32)
                nc.sync.dma_start(out=rd, in_=r)
                rb = smalls.tile([DH, QTW], F32)
                nc.sync.dma_start(out=rb, in_=rd.to_broadcast((DH, QTW)))
                nc.vector.tensor_mul(
                    out=oT[r0 : r0 + DH, h // 2, qt * QTW : (qt + 1) * QTW],
                    in0=oc[0:DH, :],
                    in1=rb,
                )

        # PE observes the final oT write tick before the out-projection
        pe_observe(oT[0:DH, 1, N - 1 : N])

        # ---- Phase D: out projection -----------------------------------
        ob_hist = []
        for tt in range(NT):
            if len(ob_hist) >= 2:
                pe_observe(ob_hist[-2][:, 0:1])
            ps = psA.tile([P, D], F32)
            for ki in range(2):
                nc.tensor.matmul(
                    ps,
                    oT[:, ki, tt * P : (tt + 1) * P],
                    wout_sb[:, ki, :],
                    start=(ki == 0),
                    stop=(ki == 1),
                )
            ob = outp.tile([P, D], F32)
            nc.vector.tensor_copy(out=ob, in_=ps)
            ob_hist.append(ob)
            nc.sync.dma_start(out=out_d[tt * P : (tt + 1) * P, :], in_=ob)

    nc.compile()
    return nc


_NC_CACHE = {}
last_results = None  # BassKernelResults of the most recent run (for test.py)


def _get_nc():
    key = _MM_DT_NAME
    if key not in _NC_CACHE:
        _NC_CACHE[key] = _build_nc()
    return _NC_CACHE[key]


def kernel(x, gamma, beta, w_qkv, w_out):
    global last_results
    x = np.ascontiguousarray(np.asarray(x, dtype=np.float32))
    gamma = np.asarray(gamma, dtype=np.float32)
    beta = np.asarray(beta, dtype=np.float32)
    w_qkv = np.asarray(w_qkv, dtype=np.float32)
    w_out = np.asarray(w_out, dtype=np.float32)

    # fold gamma/beta into the projection (exact algebra)
    wp = gamma[:, None] * w_qkv                      # [512, 1536]
    bp = beta @ w_qkv                                # [1536]

    in_maps = []
    for c in range(N_CORES):
        b = c // 2
        g = c % 2
        sl = [slice(s * D + g * GCOLS, s * D + (g + 1) * GCOLS) for s in range(3)]
        wg = np.concatenate([wp[:, s] for s in sl], axis=1)          # [512, 768]
        bg = np.concatenate([bp[s] for s in sl])[:, None]            # [768, 1]
        wo = w_out[g * GCOLS : (g + 1) * GCOLS, :]                   # [256, 512]
        in_maps.append(
            {
                "x": np.ascontiguousarray(x[b]),
                "wqkv": np.ascontiguousarray(wg.astype(np.float32)),
                "bqkv": np.ascontiguousarray(bg.astype(np.float32)),
                "wout": np.ascontiguousarray(wo.astype(np.float32)),
            }
        )

    nc = _get_nc()
    last_results = run_bass_kernel_spmd(nc, in_maps, list(range(N_CORES)))
    outs = [m["out"] for m in last_results.results]
    out = np.stack([outs[2 * b] + outs[2 * b + 1] for b in range(B)])
    return np.ascontiguousarray(out.astype(np.float32))


# revision 36
# speedup vs baseline: 1.7832x; 1.7832x over previous
"""Fused pre-norm attention kernel for Trainium2, sharded over 8 NeuronCores.

Problem: out = (LayerNorm(x) @ w_qkv -> multi-head attention) @ w_out
  x [4, 2048, 512], 8 heads x 64 dim, fp32.

Sharding: core c computes batch b = c//2 with head group g = c%2 (4 heads).
Each core produces a partial output [2048, 512] (its heads' contribution to
the out-projection); the host sums the two partials per batch.

Per-core kernel (all SBUF-resident, flash-style, no score materialization
to HBM):
  1. LayerNorm x_b in token-major layout (bn_stats), gamma/beta folded into
     the QKV weights on the host.
  2. PE-transpose xn -> xnT [512, 2048] (feature-major).
  3. QKV: qT,kT = w'.T @ xnT (feature-contraction, weights as lhsT);
     v in token-major via xnT as lhsT. A ones-column is appended to v per
     head so the P@V matmul also produces the softmax denominator.
  4. Attention in S^T layout: S^T[k,q] = K @ Q^T block; exp on ACT engine
     (scale folded in, no max subtraction -- scores are bounded ~|9|);
     O^T[d,q] accumulates over key blocks on PE. Row 64 of the O^T psum is
     the softmax denominator; divide via reciprocal + partition-broadcast.
  5. Out-projection: partial = O^T.T @ w_out_rows, DMA to DRAM.
"""

import os
import sys
from contextlib import ExitStack

import numpy as np

for _p in ("/opt/trn_rl_repo",):
    if _p not in sys.path and os.path.isdir(_p):
        sys.path.insert(0, _p)

import concourse.bacc as bacc
import concourse.bass as bass
import concourse.mybir as mybir
import concourse.tile as tile
from concourse.bass_utils import run_bass_kernel_spmd
from concourse.masks import make_identity

F32 = mybir.dt.float32
F32R = mybir.dt.float32r
AF = mybir.ActivationFunctionType

N_CORES = 8
B, N, D = 4, 2048, 512
H_PER_CORE = 4
DH = 64
GCOLS = H_PER_CORE * DH          # 256 columns per head-group
WCOLS = 3 * GCOLS                # 768 qkv columns per core
SCALE = DH ** -0.5
EPS = 1e-5
P = 128                          # SBUF partitions
NT = N // P                      # 16 token tiles
KT = D // P                      # 4 feature (contraction) tiles
QTW = 512                        # query-slice width for attention
NQT = N // QTW                   # 4 query slices

# matmul streaming dtype: float32r = fp32 data on the fast (1 cycle/row)
# PE path; float32 = exact but 4 cycles/row.
_MM_DT_NAME = os.environ.get("BASS_MM_DT", "f32r")
MM_DT = F32R if _MM_DT_NAME == "f32r" else F32
# attention-pipeline dtype (q/k/v tiles and exp(S) tiles): bf16 streams at
# 1 cycle/row on the PE vs 2 for f32r, and avoids the f32r rounding pass
# on the ACT engine's exp output.
_AT_DT_NAME = os.environ.get("BASS_AT_DT", "bf16")
AT_DT = mybir.dt.bfloat16 if _AT_DT_NAME == "bf16" else MM_DT


def _build_nc():
    nc = bacc.Bacc(None)
    x_d = nc.declare_dram_parameter("x", [N, D], F32, isOutput=False)
    wqkv_d = nc.declare_dram_parameter("wqkv", [D, WCOLS], MM_DT, isOutput=False)
    bqkv_d = nc.declare_dram_parameter("bqkv", [WCOLS, 1], F32, isOutput=False)
    wout_d = nc.declare_dram_parameter("wout", [GCOLS, D], MM_DT, isOutput=False)
    out_d = nc.declare_dram_parameter("out", [N, D], F32, isOutput=True)

    with tile.TileContext(nc, pool_alloc_mode="queue") as tc, ExitStack() as ctx:
        singles = ctx.enter_context(tc.tile_pool(name="singles", bufs=1))
        xin = ctx.enter_context(tc.tile_pool(name="xin", bufs=8))
        stats = ctx.enter_context(tc.tile_pool(name="stats", bufs=4))
        pP = ctx.enter_context(tc.tile_pool(name="pP", bufs=4))
        smalls = ctx.enter_context(tc.tile_pool(name="smalls", bufs=8))
        outp = ctx.enter_context(tc.tile_pool(name="outp", bufs=3))
        psA = ctx.enter_context(tc.tile_pool(name="psA", bufs=4, space="PSUM"))
        psS = ctx.enter_context(tc.tile_pool(name="psS", bufs=2, space="PSUM"))
        dscr = ctx.enter_context(tc.tile_pool(name="dscr", bufs=6, space="DRAM"))

        ident = singles.tile([P, P], F32)
        make_identity(nc, ident)
        eps_sb = singles.tile([P, 1], F32)
        nc.vector.memset(eps_sb, EPS)

        # persistent SBUF tensors
        xT = singles.tile([P, KT, N], MM_DT)            # xn^T  [feat, token]
        qkT = singles.tile([P, 4, N], AT_DT)            # [qT(2 tiles), kT(2 tiles)]
        v_aug = singles.tile([P, NT, H_PER_CORE, DH + 1], AT_DT)
        oT = singles.tile([P, 2, N], MM_DT)             # O^T rows (4 heads x 64)
        w_sb = singles.tile([P, KT, WCOLS], MM_DT)
        bias_sb = singles.tile([P, 6], F32)
        oc_all = singles.tile([DH + 1, NQT * H_PER_CORE, QTW], F32)
        vbias_sb = singles.tile([P, GCOLS], F32)
        wout_sb = singles.tile([P, 2, D], MM_DT)

        nc.sync.dma_start(out=w_sb, in_=wqkv_d[:, :].rearrange("(kt p) m -> p kt m", p=P))
        nc.sync.dma_start(out=bias_sb, in_=bqkv_d[:, :].rearrange("(t p) o -> p (t o)", p=P))
        nc.sync.dma_start(out=wout_sb, in_=wout_d[:, :].rearrange("(ki p) n -> p ki n", p=P))
        bq = bqkv_d[:, :]
        vbias_bcast = bass.AP(
            tensor=bq.tensor, offset=2 * GCOLS, ap=[[0, P], [1, GCOLS]]
        )
        nc.sync.dma_start(out=vbias_sb, in_=vbias_bcast)

        # ones columns of v_aug (f32r memset is not a valid ISA op; copy
        # from an f32 ones tile instead -- DVE rounds on write)
        ones_sb = singles.tile([P, 1], F32)
        nc.vector.memset(ones_sb, 1.0)
        nc.vector.tensor_copy(
            out=v_aug[:, :, :, DH : DH + 1],
            in_=ones_sb.to_broadcast((P, NT, H_PER_CORE, 1)),
        )

        # PE matmuls accept only ONE sync wait command (walrus
        # setupSyncWait on the S3_LW format). Sacrificial ldweights ops
        # (no PSUM output, single dependency each) make the PE observe
        # fresh semaphore ticks so real matmuls keep to one wait.
        BF16 = mybir.dt.bfloat16

        def pe_observe(ap):
            nc.tensor.ldweights(ap.bitcast(BF16))

        pe_observe(ident[:, 0:1])
        pe_observe(w_sb[:, 0, 0:1])
        pe_observe(wout_sb[:, 0, 0:1])

        # ---- Phase A: LayerNorm + transpose ----------------------------
        for tt in range(NT):
            x_tile = xin.tile([P, D], F32)
            nc.sync.dma_start(out=x_tile, in_=x_d[tt * P : (tt + 1) * P, :])
            st = stats.tile([P, nc.vector.BN_STATS_DIM], F32)
            nc.vector.bn_stats(out=st, in_=x_tile)
            mv = stats.tile([P, nc.vector.BN_AGGR_DIM], F32)
            nc.vector.bn_aggr(out=mv, in_=st)
            rstd = stats.tile([P, 1], F32)
            nc.scalar.activation(out=rstd, in_=mv[:, 1:2], func=AF.Sqrt, bias=eps_sb)
            nc.vector.reciprocal(out=rstd, in_=rstd)
            nc.vector.tensor_scalar(
                out=x_tile,
                in0=x_tile,
                scalar1=mv[:, 0:1],
                scalar2=rstd,
                op0=mybir.AluOpType.subtract,
                op1=mybir.AluOpType.mult,
            )
            for ft in range(KT):
                ps = psA.tile([P, P], F32)
                nc.tensor.transpose(ps, x_tile[:, ft * P : (ft + 1) * P], ident)
                nc.scalar.copy(out=xT[:, ft, tt * P : (tt + 1) * P], in_=ps)
            # v projection for this token tile (its xT slices just landed)
            if tt >= 2:
                pe_observe(v_aug[:, tt - 2, 0, 0:1])
            ps = psA.tile([P, GCOLS], F32)
            for kt in range(KT):
                nc.tensor.matmul(
                    ps,
                    xT[:, kt, tt * P : (tt + 1) * P],
                    w_sb[:, kt, 2 * GCOLS : 3 * GCOLS],
                    start=(kt == 0),
                    stop=(kt == KT - 1),
                )
            nc.vector.tensor_add(
                out=v_aug[:, tt, :, 0:DH],
                in0=ps.rearrange("p (h d) -> p h d", h=H_PER_CORE),
                in1=vbias_sb.rearrange("p (h d) -> p h d", h=H_PER_CORE),
            )

        # PE observes the final xT copy tick before QKV matmuls
        pe_observe(xT[:, KT - 1, N - 1 : N])

        # ---- Phase B: q/k projections for head-pair 0 ------------------
        # Only mi 0 (q heads 0-1) and mi 2 (k heads 0-1) are projected
        # upfront; mi 1/3 (head-pair 1) are deferred into the attention
        # loop below, filling PE slack while ACT runs exps.
        def qk_group_steps(mi, nt):
            ps = psA.tile([P, QTW], F32, tag="ps")
            for kt in range(KT):
                yield lambda kt=kt, ps=ps: nc.tensor.matmul(
                    ps,
                    w_sb[:, kt, mi * P : (mi + 1) * P],
                    xT[:, kt, nt * QTW : (nt + 1) * QTW],
                    start=(kt == 0),
                    stop=(kt == KT - 1),
                )
            yield lambda ps=ps: nc.vector.tensor_scalar(
                out=qkT[:, mi, nt * QTW : (nt + 1) * QTW],
                in0=ps,
                scalar1=bias_sb[:, mi : mi + 1],
                scalar2=None,
                op0=mybir.AluOpType.add,
            )

        for mi in (0, 2):
            for nt in range(NQT):
                for step in qk_group_steps(mi, nt):
                    step()

        def deferred_qk():
            for mi in (1, 3):
                for nt in range(NQT):
                    yield from qk_group_steps(mi, nt)

        _deferred = deferred_qk()

        # PE observes the final upfront qkT write (covers v_aug too)
        pe_observe(qkT[:, 2, N - 1 : N])
        last_flush = {}

        # ---- Phase C: attention (S^T layout, head-paired) --------------
        # The two heads of each qkT M-tile occupy partition rows 0-63 and
        # 64-127, so their K=64 S^T matmuls run CONCURRENTLY on the PE via
        # row-group packing, share one [128, 1024] PSUM tile, and are
        # consumed by a single 1024-wide exp on ACT (halving its
        # per-instruction overhead).
        for hp in range(2):
            h0, h1 = 2 * hp, 2 * hp + 1
            mi_q, mi_k = hp, 2 + hp
            for qt in range(NQT):
                qs = slice(qt * QTW, (qt + 1) * QTW)
                po0 = psA.tile([DH + 1, QTW], F32, tag="ps")
                po1 = psA.tile([DH + 1, QTW], F32, tag="ps")
                pending = None
                for kb in range(NT):
                    ks = slice(kb * P, (kb + 1) * P)
                    ps_s = psS.tile([P, 2, QTW], F32)
                    nc.tensor.matmul(
                        ps_s[:, 0, :],
                        qkT[0:DH, mi_k, ks],
                        qkT[0:DH, mi_q, qs],
                        start=True,
                        stop=True,
                    )
                    nc.tensor.matmul(
                        ps_s[:, 1, :],
                        qkT[DH:P, mi_k, ks],
                        qkT[DH:P, mi_q, qs],
                        start=True,
                        stop=True,
                    )
                    pT = pP.tile([P, 2, QTW], AT_DT)
                    nc.scalar.activation(out=pT, in_=ps_s, func=AF.Exp, scale=SCALE)
                    if pending is not None:
                        pkb, ppT = pending
                        if pkb == 0:
                            pe_observe(ppT[:, 0, 0:1])
                        nc.tensor.matmul(
                            po0, v_aug[:, pkb, h0, :], ppT[:, 0, :],
                            start=(pkb == 0), stop=False,
                        )
                        nc.tensor.matmul(
                            po1, v_aug[:, pkb, h1, :], ppT[:, 1, :],
                            start=(pkb == 0), stop=False,
                        )
                        if hp == 0:
                            # one deferred q/k projection step per slot,
                            # consuming attention-phase PE slack
                            step = next(_deferred, None)
                            if step is not None:
                                step()
                    pending = (kb, pT)
                pkb, ppT = pending
                nc.tensor.matmul(
                    po0, v_aug[:, pkb, h0, :], ppT[:, 0, :], start=False, stop=True,
                )
                last_att_mm = nc.tensor.matmul(
                    po1, v_aug[:, pkb, h1, :], ppT[:, 1, :], start=False, stop=True,
                )
                if hp == 1:
                    last_flush[qt] = last_att_mm
                # normalize both heads: fast ACT copies release the PSUM
                # slots promptly; the recip/DMA/mul chain lags by design and
                # only gates phase D (pinned after attention).
                for h, po in ((h0, po0), (h1, po1)):
                    u = h * NQT + qt
                    r0 = (h % 2) * DH
                    nc.vector.tensor_copy(out=oc_all[:, u, :], in_=po)
                    r = smalls.tile([1, QTW], F32)
                    nc.vector.reciprocal(out=r, in_=oc_all[DH : DH + 1, u, :])
                    rd = dscr.tile([1, QTW], F32)
                    nc.sync.dma_start(out=rd, in_=r)
                    rb = smalls.tile([DH, QTW], F32)
                    nc.sync.dma_start(out=rb, in_=rd.to_broadcast((DH, QTW)))
                    nc.vector.tensor_mul(
                        out=oT[r0 : r0 + DH, h // 2, qs],
                        in0=oc_all[0:DH, u, :],
                        in1=rb,
                    )

        # Keep the PE HAM-warm across the normalize-chain tail so the
        # out-projection runs at full clock: a short burst of
        # dependency-free matmuls into a scratch psum slot.
        for wk in range(12):
            ps = psA.tile([P, QTW], F32, tag="ps")
            nc.tensor.matmul(
                ps,
                qkT[0:DH, 0, 0:P],
                qkT[0:DH, 0, 0:QTW],
                start=True,
                stop=True,
            )

        # PE observes the final oT write tick before the out-projection
        pe_observe(oT[0:DH, 1, N - 1 : N])

        # ---- Phase D: out projection -----------------------------------
        ob_hist = []
        for tt in range(NT):
            if len(ob_hist) >= 2:
                pe_observe(ob_hist[-2][:, 0:1])
            ps = psA.tile([P, D], F32)
            for ki in range(2):
                mm = nc.tensor.matmul(
                    ps,
                    oT[:, ki, tt * P : (tt + 1) * P],
                    wout_sb[:, ki, :],
                    start=(ki == 0),
                    stop=(ki == 1),
                )
                # pin on the flush of query-slice tt//4 + 1: by then the
                # lazy normalize chain for tt//4 has certainly completed,
                # so this matmul never blocks the PE queue mid-attention,
                # yet D overlaps the last attention units.
                pin = last_flush[min(tt // NQT + 1, NQT - 1)]
                tile.add_dep_helper(
                    mm.ins, pin.ins, sync=False,
                    reason="phase D after covering attention flush",
                )
            ob = outp.tile([P, D], F32)
            nc.vector.tensor_copy(out=ob, in_=ps)
            ob_hist.append(ob)
            nc.sync.dma_start(out=out_d[tt * P : (tt + 1) * P, :], in_=ob)

    nc.compile()
    return nc


_NC_CACHE = {}
last_results = None  # BassKernelResults of the most recent run (for test.py)


def _get_nc():
    key = (_MM_DT_NAME, _AT_DT_NAME)
    if key not in _NC_CACHE:
        _NC_CACHE[key] = _build_nc()
    return _NC_CACHE[key]


def kernel(x, gamma, beta, w_qkv, w_out):
    global last_results
    x = np.ascontiguousarray(np.asarray(x, dtype=np.float32))
    gamma = np.asarray(gamma, dtype=np.float32)
    beta = np.asarray(beta, dtype=np.float32)
    w_qkv = np.asarray(w_qkv, dtype=np.float32)
    w_out = np.asarray(w_out, dtype=np.float32)

    # fold gamma/beta into the projection (exact algebra)
    wp = gamma[:, None] * w_qkv                      # [512, 1536]
    bp = beta @ w_qkv                                # [1536]

    in_maps = []
    for c in range(N_CORES):
        b = c // 2
        g = c % 2
        sl = [slice(s * D + g * GCOLS, s * D + (g + 1) * GCOLS) for s in range(3)]
        wg = np.concatenate([wp[:, s] for s in sl], axis=1)          # [512, 768]
        bg = np.concatenate([bp[s] for s in sl])[:, None]            # [768, 1]
        wo = w_out[g * GCOLS : (g + 1) * GCOLS, :]                   # [256, 512]
        in_maps.append(
            {
                "x": np.ascontiguousarray(x[b]),
                "wqkv": np.ascontiguousarray(wg.astype(np.float32)),
                "bqkv": np.ascontiguousarray(bg.astype(np.float32)),
                "wout": np.ascontiguousarray(wo.astype(np.float32)),
            }
        )

    nc = _get_nc()
    last_results = run_bass_kernel_spmd(nc, in_maps, list(range(N_CORES)))
    outs = [m["out"] for m in last_results.results]
    out = np.stack([outs[2 * b] + outs[2 * b + 1] for b in range(B)])
    return np.ascontiguousarray(out.astype(np.float32))


# revision 39
# speedup vs baseline: 1.8106x; 1.0154x over previous
"""Fused pre-norm attention kernel for Trainium2, sharded over 8 NeuronCores.

Problem: out = (LayerNorm(x) @ w_qkv -> multi-head attention) @ w_out
  x [4, 2048, 512], 8 heads x 64 dim, fp32.

Sharding: core c computes batch b = c//2 with head group g = c%2 (4 heads).
Each core produces a partial output [2048, 512] (its heads' contribution to
the out-projection); the host sums the two partials per batch.

Per-core kernel (all SBUF-resident, flash-style, no score materialization
to HBM; measured ~230us HW exec across 8 cores, rel err ~3.5e-3):
  1. LayerNorm x_b token-major (bn_stats), gamma/beta folded into the QKV
     weights on the host; PE-transpose xn -> xnT [512, 2048]; the
     v-projection runs per token tile inside this loop. A ones-column is
     appended to v per head so P@V also produces the softmax denominator.
  2. q/k projections for head-pair 0 upfront; head-pair 1's are deferred
     into the attention loop, one matmul per key-block, filling PE slack
     while ACT runs exps.
  3. Attention in S^T layout, head-paired: the two heads of a qkT M-tile
     occupy partition rows 0-63/64-127, so their K=64 S^T = K @ Q^T
     matmuls run concurrently (PE row-group packing) into one
     [128, 2, 512] PSUM tile, consumed by a single 1024-wide exp on ACT
     (scale folded in; no max subtraction -- scores bounded ~|9|).
     O^T accumulates per head over key blocks on the PE (bf16 pipeline).
  4. Softmax normalization: row 64 of each O^T psum holds the denominator;
     a fast DVE staging copy releases the PSUM slot, then a lazy
     reciprocal + DRAM-bounce partition-broadcast + multiply chain writes
     normalized O^T (f32r) without ever blocking the PE queue.
  5. Out-projection: partial = O^T.T @ w_out_rows; each token tile is
     pinned (no-sync dep) on a later attention flush so it overlaps late
     attention without stalling it; warm-keeper matmuls bridge the HAM
     clock gate across the normalize tail.
"""

import os
import sys
from contextlib import ExitStack

import numpy as np

for _p in ("/opt/trn_rl_repo",):
    if _p not in sys.path and os.path.isdir(_p):
        sys.path.insert(0, _p)

import concourse.bacc as bacc
import concourse.bass as bass
import concourse.mybir as mybir
import concourse.tile as tile
from concourse.bass_utils import run_bass_kernel_spmd
from concourse.masks import make_identity

F32 = mybir.dt.float32
F32R = mybir.dt.float32r
AF = mybir.ActivationFunctionType

N_CORES = 8
B, N, D = 4, 2048, 512
H_PER_CORE = 4
DH = 64
GCOLS = H_PER_CORE * DH          # 256 columns per head-group
WCOLS = 3 * GCOLS                # 768 qkv columns per core
SCALE = DH ** -0.5
EPS = 1e-5
P = 128                          # SBUF partitions
NT = N // P                      # 16 token tiles
KT = D // P                      # 4 feature (contraction) tiles
QTW = 512                        # query-slice width for attention
NQT = N // QTW                   # 4 query slices

# matmul streaming dtype: float32r = fp32 data on the fast (1 cycle/row)
# PE path; float32 = exact but 4 cycles/row.
_MM_DT_NAME = os.environ.get("BASS_MM_DT", "f32r")
MM_DT = F32R if _MM_DT_NAME == "f32r" else F32
# attention-pipeline dtype (q/k/v tiles and exp(S) tiles): bf16 streams at
# 1 cycle/row on the PE vs 2 for f32r, and avoids the f32r rounding pass
# on the ACT engine's exp output.
_AT_DT_NAME = os.environ.get("BASS_AT_DT", "bf16")
AT_DT = mybir.dt.bfloat16 if _AT_DT_NAME == "bf16" else MM_DT


def _build_nc():
    nc = bacc.Bacc(None)
    x_d = nc.declare_dram_parameter("x", [N, D], F32, isOutput=False)
    wqkv_d = nc.declare_dram_parameter("wqkv", [D, WCOLS], MM_DT, isOutput=False)
    bqkv_d = nc.declare_dram_parameter("bqkv", [WCOLS, 1], F32, isOutput=False)
    wout_d = nc.declare_dram_parameter("wout", [GCOLS, D], MM_DT, isOutput=False)
    out_d = nc.declare_dram_parameter("out", [N, D], F32, isOutput=True)

    with tile.TileContext(nc, pool_alloc_mode="queue") as tc, ExitStack() as ctx:
        singles = ctx.enter_context(tc.tile_pool(name="singles", bufs=1))
        xin = ctx.enter_context(tc.tile_pool(name="xin", bufs=8))
        stats = ctx.enter_context(tc.tile_pool(name="stats", bufs=4))
        pP = ctx.enter_context(tc.tile_pool(name="pP", bufs=4))
        smalls = ctx.enter_context(tc.tile_pool(name="smalls", bufs=8))
        outp = ctx.enter_context(tc.tile_pool(name="outp", bufs=3))
        psA = ctx.enter_context(tc.tile_pool(name="psA", bufs=4, space="PSUM"))
        psS = ctx.enter_context(tc.tile_pool(name="psS", bufs=2, space="PSUM"))
        dscr = ctx.enter_context(tc.tile_pool(name="dscr", bufs=6, space="DRAM"))

        ident = singles.tile([P, P], F32)
        make_identity(nc, ident)
        eps_sb = singles.tile([P, 1], F32)
        nc.vector.memset(eps_sb, EPS)

        # persistent SBUF tensors
        xT = singles.tile([P, KT, N], MM_DT)            # xn^T  [feat, token]
        qkT = singles.tile([P, 4, N], AT_DT)            # [qT(2 tiles), kT(2 tiles)]
        v_aug = singles.tile([P, NT, H_PER_CORE, DH + 1], AT_DT)
        oT = singles.tile([P, 2, N], MM_DT)             # O^T rows (4 heads x 64)
        w_sb = singles.tile([P, KT, WCOLS], MM_DT)
        bias_sb = singles.tile([P, 6], F32)
        oc_all = singles.tile([DH + 1, NQT * H_PER_CORE, QTW], F32)
        vbias_sb = singles.tile([P, GCOLS], F32)
        wout_sb = singles.tile([P, 2, D], MM_DT)

        nc.sync.dma_start(out=w_sb, in_=wqkv_d[:, :].rearrange("(kt p) m -> p kt m", p=P))
        nc.sync.dma_start(out=bias_sb, in_=bqkv_d[:, :].rearrange("(t p) o -> p (t o)", p=P))
        nc.sync.dma_start(out=wout_sb, in_=wout_d[:, :].rearrange("(ki p) n -> p ki n", p=P))
        bq = bqkv_d[:, :]
        vbias_bcast = bass.AP(
            tensor=bq.tensor, offset=2 * GCOLS, ap=[[0, P], [1, GCOLS]]
        )
        nc.sync.dma_start(out=vbias_sb, in_=vbias_bcast)

        # ones columns of v_aug (f32r memset is not a valid ISA op; copy
        # from an f32 ones tile instead -- DVE rounds on write)
        ones_sb = singles.tile([P, 1], F32)
        nc.vector.memset(ones_sb, 1.0)
        nc.vector.tensor_copy(
            out=v_aug[:, :, :, DH : DH + 1],
            in_=ones_sb.to_broadcast((P, NT, H_PER_CORE, 1)),
        )

        # PE matmuls accept only ONE sync wait command (walrus
        # setupSyncWait on the S3_LW format). Sacrificial ldweights ops
        # (no PSUM output, single dependency each) make the PE observe
        # fresh semaphore ticks so real matmuls keep to one wait.
        BF16 = mybir.dt.bfloat16

        def pe_observe(ap):
            nc.tensor.ldweights(ap.bitcast(BF16))

        pe_observe(ident[:, 0:1])
        pe_observe(w_sb[:, 0, 0:1])
        pe_observe(wout_sb[:, 0, 0:1])

        # ---- Phase A: LayerNorm + transpose ----------------------------
        for tt in range(NT):
            x_tile = xin.tile([P, D], F32)
            nc.sync.dma_start(out=x_tile, in_=x_d[tt * P : (tt + 1) * P, :])
            st = stats.tile([P, nc.vector.BN_STATS_DIM], F32)
            nc.vector.bn_stats(out=st, in_=x_tile)
            mv = stats.tile([P, nc.vector.BN_AGGR_DIM], F32)
            nc.vector.bn_aggr(out=mv, in_=st)
            rstd = stats.tile([P, 1], F32)
            nc.scalar.activation(out=rstd, in_=mv[:, 1:2], func=AF.Sqrt, bias=eps_sb)
            nc.vector.reciprocal(out=rstd, in_=rstd)
            nc.vector.tensor_scalar(
                out=x_tile,
                in0=x_tile,
                scalar1=mv[:, 0:1],
                scalar2=rstd,
                op0=mybir.AluOpType.subtract,
                op1=mybir.AluOpType.mult,
            )
            for ft in range(KT):
                ps = psA.tile([P, P], F32)
                nc.tensor.transpose(ps, x_tile[:, ft * P : (ft + 1) * P], ident)
                nc.scalar.copy(out=xT[:, ft, tt * P : (tt + 1) * P], in_=ps)
            # v projection for this token tile (its xT slices just landed)
            if tt >= 2:
                pe_observe(v_aug[:, tt - 2, 0, 0:1])
            ps = psA.tile([P, GCOLS], F32)
            for kt in range(KT):
                nc.tensor.matmul(
                    ps,
                    xT[:, kt, tt * P : (tt + 1) * P],
                    w_sb[:, kt, 2 * GCOLS : 3 * GCOLS],
                    start=(kt == 0),
                    stop=(kt == KT - 1),
                )
            nc.vector.tensor_add(
                out=v_aug[:, tt, :, 0:DH],
                in0=ps.rearrange("p (h d) -> p h d", h=H_PER_CORE),
                in1=vbias_sb.rearrange("p (h d) -> p h d", h=H_PER_CORE),
            )

        # PE observes the final xT copy tick before QKV matmuls
        pe_observe(xT[:, KT - 1, N - 1 : N])

        # ---- Phase B: q/k projections for head-pair 0 ------------------
        # Only mi 0 (q heads 0-1) and mi 2 (k heads 0-1) are projected
        # upfront; mi 1/3 (head-pair 1) are deferred into the attention
        # loop below, filling PE slack while ACT runs exps.
        def qk_group_steps(mi, nt):
            ps = psA.tile([P, QTW], F32, tag="ps")
            for kt in range(KT):
                yield lambda kt=kt, ps=ps: nc.tensor.matmul(
                    ps,
                    w_sb[:, kt, mi * P : (mi + 1) * P],
                    xT[:, kt, nt * QTW : (nt + 1) * QTW],
                    start=(kt == 0),
                    stop=(kt == KT - 1),
                )
            yield lambda ps=ps: nc.vector.tensor_scalar(
                out=qkT[:, mi, nt * QTW : (nt + 1) * QTW],
                in0=ps,
                scalar1=bias_sb[:, mi : mi + 1],
                scalar2=None,
                op0=mybir.AluOpType.add,
            )

        for mi in (0, 2):
            for step in qk_group_steps(mi, 0):
                step()

        def steps_of(groups):
            for mi, nt in groups:
                yield from qk_group_steps(mi, nt)

        # remaining head-pair-0 groups stream into unit (0,0)'s key-block
        # loop at 2 steps/block: each kT slice (mi=2, nt) completes just
        # before the S matmuls for key blocks 4nt..4nt+3 consume it.
        _early = steps_of([(2, 1), (2, 2), (2, 3), (0, 1), (0, 2), (0, 3)])
        _deferred = steps_of([(1, nt) for nt in range(NQT)]
                             + [(3, nt) for nt in range(NQT)])

        # PE observes the final upfront qkT write (covers v_aug too)
        pe_observe(qkT[:, 2, QTW - 1 : QTW])
        last_flush = {}

        # ---- Phase C: attention (S^T layout, head-paired) --------------
        # The two heads of each qkT M-tile occupy partition rows 0-63 and
        # 64-127, so their K=64 S^T matmuls run CONCURRENTLY on the PE via
        # row-group packing, share one [128, 1024] PSUM tile, and are
        # consumed by a single 1024-wide exp on ACT (halving its
        # per-instruction overhead).
        for hp in range(2):
            h0, h1 = 2 * hp, 2 * hp + 1
            mi_q, mi_k = hp, 2 + hp
            for qt in range(NQT):
                qs = slice(qt * QTW, (qt + 1) * QTW)
                po0 = psA.tile([DH + 1, QTW], F32, tag="ps")
                po1 = psA.tile([DH + 1, QTW], F32, tag="ps")
                pending = None
                for kb in range(NT):
                    ks = slice(kb * P, (kb + 1) * P)
                    ps_s = psS.tile([P, 2, QTW], F32)
                    nc.tensor.matmul(
                        ps_s[:, 0, :],
                        qkT[0:DH, mi_k, ks],
                        qkT[0:DH, mi_q, qs],
                        start=True,
                        stop=True,
                    )
                    nc.tensor.matmul(
                        ps_s[:, 1, :],
                        qkT[DH:P, mi_k, ks],
                        qkT[DH:P, mi_q, qs],
                        start=True,
                        stop=True,
                    )
                    pT = pP.tile([P, 2, QTW], AT_DT)
                    nc.scalar.activation(out=pT, in_=ps_s, func=AF.Exp, scale=SCALE)
                    if pending is not None:
                        pkb, ppT = pending
                        if pkb == 0:
                            pe_observe(ppT[:, 0, 0:1])
                        nc.tensor.matmul(
                            po0, v_aug[:, pkb, h0, :], ppT[:, 0, :],
                            start=(pkb == 0), stop=False,
                        )
                        nc.tensor.matmul(
                            po1, v_aug[:, pkb, h1, :], ppT[:, 1, :],
                            start=(pkb == 0), stop=False,
                        )
                        if hp == 0 and qt == 0:
                            # stream the remaining head-pair-0 projections
                            for _ in range(2):
                                step = next(_early, None)
                                if step is not None:
                                    step()
                        elif hp == 0:
                            step = next(_early, None) or next(_deferred, None)
                            if step is not None:
                                step()
                    pending = (kb, pT)
                pkb, ppT = pending
                nc.tensor.matmul(
                    po0, v_aug[:, pkb, h0, :], ppT[:, 0, :], start=False, stop=True,
                )
                last_att_mm = nc.tensor.matmul(
                    po1, v_aug[:, pkb, h1, :], ppT[:, 1, :], start=False, stop=True,
                )
                if hp == 1:
                    last_flush[qt] = last_att_mm
                # normalize both heads: fast ACT copies release the PSUM
                # slots promptly; the recip/DMA/mul chain lags by design and
                # only gates phase D (pinned after attention).
                for h, po in ((h0, po0), (h1, po1)):
                    u = h * NQT + qt
                    r0 = (h % 2) * DH
                    nc.vector.tensor_copy(out=oc_all[:, u, :], in_=po)
                    r = smalls.tile([1, QTW], F32)
                    nc.vector.reciprocal(out=r, in_=oc_all[DH : DH + 1, u, :])
                    rd = dscr.tile([1, QTW], F32)
                    nc.sync.dma_start(out=rd, in_=r)
                    rb = smalls.tile([DH, QTW], F32)
                    nc.sync.dma_start(out=rb, in_=rd.to_broadcast((DH, QTW)))
                    nc.vector.tensor_mul(
                        out=oT[r0 : r0 + DH, h // 2, qs],
                        in0=oc_all[0:DH, u, :],
                        in1=rb,
                    )

        # Keep the PE HAM-warm across the normalize-chain tail so the
        # out-projection runs at full clock: a short burst of
        # dependency-free matmuls into a scratch psum slot.
        for wk in range(12):
            ps = psA.tile([P, QTW], F32, tag="ps")
            nc.tensor.matmul(
                ps,
                qkT[0:DH, 0, 0:P],
                qkT[0:DH, 0, 0:QTW],
                start=True,
                stop=True,
            )

        # PE observes the final oT write tick before the out-projection
        pe_observe(oT[0:DH, 1, N - 1 : N])

        # ---- Phase D: out projection -----------------------------------
        ob_hist = []
        for tt in range(NT):
            if len(ob_hist) >= 2:
                pe_observe(ob_hist[-2][:, 0:1])
            ps = psA.tile([P, D], F32)
            for ki in range(2):
                mm = nc.tensor.matmul(
                    ps,
                    oT[:, ki, tt * P : (tt + 1) * P],
                    wout_sb[:, ki, :],
                    start=(ki == 0),
                    stop=(ki == 1),
                )
                # pin on the flush of query-slice tt//4 + 1: by then the
                # lazy normalize chain for tt//4 has certainly completed,
                # so this matmul never blocks the PE queue mid-attention,
                # yet D overlaps the last attention units.
                pin = last_flush[min(tt // NQT + 1, NQT - 1)]
                tile.add_dep_helper(
                    mm.ins, pin.ins, sync=False,
                    reason="phase D after covering attention flush",
                )
            ob = outp.tile([P, D], F32)
            nc.vector.tensor_copy(out=ob, in_=ps)
            ob_hist.append(ob)
            nc.sync.dma_start(out=out_d[tt * P : (tt + 1) * P, :], in_=ob)

    nc.compile()
    return nc


_NC_CACHE = {}
last_results = None  # BassKernelResults of the most recent run (for test.py)


def _get_nc():
    key = (_MM_DT_NAME, _AT_DT_NAME)
    if key not in _NC_CACHE:
        _NC_CACHE[key] = _build_nc()
    return _NC_CACHE[key]


def kernel(x, gamma, beta, w_qkv, w_out):
    global last_results
    x = np.ascontiguousarray(np.asarray(x, dtype=np.float32))
    gamma = np.asarray(gamma, dtype=np.float32)
    beta = np.asarray(beta, dtype=np.float32)
    w_qkv = np.asarray(w_qkv, dtype=np.float32)
    w_out = np.asarray(w_out, dtype=np.float32)

    # fold gamma/beta into the projection (exact algebra)
    wp = gamma[:, None] * w_qkv                      # [512, 1536]
    bp = beta @ w_qkv                                # [1536]

    in_maps = []
    for c in range(N_CORES):
        b = c // 2
        g = c % 2
        sl = [slice(s * D + g * GCOLS, s * D + (g + 1) * GCOLS) for s in range(3)]
        wg = np.concatenate([wp[:, s] for s in sl], axis=1)          # [512, 768]
        bg = np.concatenate([bp[s] for s in sl])[:, None]            # [768, 1]
        wo = w_out[g * GCOLS : (g + 1) * GCOLS, :]                   # [256, 512]
        in_maps.append(
            {
                "x": np.ascontiguousarray(x[b]),
                "wqkv": np.ascontiguousarray(wg.astype(np.float32)),
                "bqkv": np.ascontiguousarray(bg.astype(np.float32)),
                "wout": np.ascontiguousarray(wo.astype(np.float32)),
            }
        )

    nc = _get_nc()
    last_results = run_bass_kernel_spmd(nc, in_maps, list(range(N_CORES)))
    outs = [m["out"] for m in last_results.results]
    out = np.stack([outs[2 * b] + outs[2 * b + 1] for b in range(B)])
    return np.ascontiguousarray(out.astype(np.float32))
